# revision 25
# baseline (speedup 1.0000x reference)
"""Trainium2 Bass kernel for an AttentionBlock:
GroupNorm(8 groups) -> 1x1 conv q/k/v -> multi-head attention (4 heads)
-> 1x1 conv proj -> residual add.

Shapes (hardcoded): x [4, 256, 64, 64]; L = 64*64 = 4096; head dim 64.

Sharding: 8 cores = (batch, query-half). Each core computes the full
GroupNorm + K/V for its batch, and attention + projection + residual for
its half (2048) of the query positions. Host permutes each batch's pixel
columns so a core's query half is always columns 0:2048 (attention is
permutation-invariant over key positions, GroupNorm over pixels), so all
8 cores run one SPMD program. No collectives; host just concatenates.

Key kernel ideas:
- S is computed transposed ([key e x query d] via lhsT=K-chunk, rhs=Q)
  so softmax needs no cross-partition reductions.
- Softmax skips the max-subtraction (logits are within [-8, 8] for
  normalized inputs; exp is exact to ~2 ULP there) -> exp(S^T) directly
  on ScalarE out of PSUM, scale=hd^-0.5 folded into the activation.
- The softmax denominator Z is produced by the PV matmul itself: V^T is
  materialized per head with a 65th column of ones, so PV output row 64
  accumulates sum_e(P) while rows 0..63 accumulate V@P.
- Attention operands (q, k, P=exp(S), V^T) are fp16 (~5e-4 quantization):
  fp16 streams 1 column/cycle on the normal PE datapath, which keeps the
  HAM clock gate warm at 2.4 GHz -- float32r runs on the transpose-mode
  path that never warms HAM, capping PE at ~1.2 GHz. Non-attention
  matmuls (GroupNorm stats, QKV, projection) stay float32r, accumulation
  is always fp32 in PSUM.
- The two heads of a pair are packed into the PE array with row tiling
  (K=64 each at rows 0-63/64-127); db is the outer loop so projection
  overlaps the next block's attention; pair-1 QKV/V^T and all softmax
  epilogues are emitted deferred so they execute under the ScalarE-bound
  exp stream (ScalarE exp of all 33.5M logits/core is the ~275us floor).
"""

import numpy as np

B, C, H, W = 4, 256, 64, 64
NH, G, EPS = 4, 8, 1e-5
L = H * W            # 4096
DH = L // 2          # query positions per core
HD = C // NH         # 64
P = 128              # SBUF partitions
CT = C // P          # channel tiles (2)
LC = L // 512        # 8 key-dim 512-chunks
DBLK = DH // 512     # 4 query-dim 512-blocks
ECH = L // P         # 32 key-dim 128-chunks
SCALE = float(HD) ** -0.5
NCORES = 8

_CACHE = {}


def _build_nc():
    import concourse.bacc as bacc
    import concourse.bass as bass
    import concourse.mybir as mybir
    import concourse.tile as tile
    from concourse.masks import make_identity
    from contextlib import ExitStack

    f32 = mybir.dt.float32
    f16 = mybir.dt.float16
    f32r = mybir.dt.float32r
    AX = mybir.AxisListType
    OP = mybir.AluOpType
    ACT = mybir.ActivationFunctionType

    def r(ap):
        return ap.bitcast(f32r)

    def mktile(pool, shape, tag, dtype=None):
        return pool.tile(shape, dtype or f32, name=tag, tag=tag)

    nc = bacc.Bacc(trn_type="TRN2", target_bir_lowering=False, num_devices=NCORES)

    x_ext = nc.declare_dram_parameter("x", [C, L], f32, isOutput=False)
    wq_ext = nc.declare_dram_parameter("wqt", [C, C], f32, isOutput=False)
    wk_ext = nc.declare_dram_parameter("wkt", [C, C], f32, isOutput=False)
    wv_ext = nc.declare_dram_parameter("wvt", [C, C], f32, isOutput=False)
    wp_ext = nc.declare_dram_parameter("wpt", [C, C], f32, isOutput=False)
    bq_ext = nc.declare_dram_parameter("bq", [C, 1], f32, isOutput=False)
    bk_ext = nc.declare_dram_parameter("bk", [C, 1], f32, isOutput=False)
    bv_ext = nc.declare_dram_parameter("bv", [C, 1], f32, isOutput=False)
    bp_ext = nc.declare_dram_parameter("bp", [C, 1], f32, isOutput=False)
    gnw_ext = nc.declare_dram_parameter("gnw", [C, 1], f32, isOutput=False)
    gnb_ext = nc.declare_dram_parameter("gnb", [C, 1], f32, isOutput=False)
    ones_ext = nc.declare_dram_parameter("ones", [P, 64], f32, isOutput=False)
    ind_ext = nc.declare_dram_parameter("ind", [C, G], f32, isOutput=False)
    indT_ext = nc.declare_dram_parameter("indT", [G, C], f32, isOutput=False)
    out_ext = nc.declare_dram_parameter("out", [C, DH], f32, isOutput=True)

    with tile.TileContext(nc) as tc, ExitStack() as top:
        # ---- pool A: kernel-long tiles -------------------------------
        pa = top.enter_context(tc.tile_pool(name="pa", bufs=1))
        wts = {}
        for nm, ext in (("wqt", wq_ext), ("wkt", wk_ext), ("wvt", wv_ext)):
            wts[nm] = []
            for ct in range(CT):
                t = mktile(pa, [P, C], f"{nm}{ct}")
                nc.gpsimd.dma_start(out=r(t[:]), in_=r(ext[ct * P:(ct + 1) * P, :]))
                wts[nm].append(t)
        # proj weight per head (base partition 0 for K=64 contraction)
        wpth = []
        for h in range(NH):
            t = mktile(pa, [HD, C], f"wpt{h}")
            nc.gpsimd.dma_start(out=r(t[:]), in_=r(wp_ext[h * HD:(h + 1) * HD, :]))
            wpth.append(t)
        bias = {}
        for nm, ext in (("bq", bq_ext), ("bk", bk_ext), ("bv", bv_ext),
                        ("bp", bp_ext), ("gnw", gnw_ext), ("gnb", gnb_ext)):
            bias[nm] = []
            for ct in range(CT):
                t = mktile(pa, [P, 1], f"{nm}{ct}")
                nc.gpsimd.dma_start(out=t[:], in_=ext[ct * P:(ct + 1) * P, :])
                bias[nm].append(t)
        indt = []
        for ct in range(CT):
            t = mktile(pa, [P, G], f"ind{ct}")
            nc.gpsimd.dma_start(out=r(t[:]), in_=r(ind_ext[ct * P:(ct + 1) * P, :]))
            indt.append(t)
        indTt = mktile(pa, [G, C], "indTt")
        nc.gpsimd.dma_start(out=indTt[:], in_=indT_ext[:])
        ident = mktile(pa, [P, P], "ident")
        make_identity(nc, ident[:])
        ones_src = mktile(pa, [P, 64], "ones_src")
        nc.gpsimd.dma_start(out=r(ones_src[:]), in_=r(ones_ext[:]))
        # attention output, one tile per head [64, DH] at base partition 0
        ao = [mktile(pa, [HD, DH], f"ao{h}") for h in range(NH)]

        # ---- pool B: normalized activations h ------------------------
        pb = top.enter_context(tc.tile_pool(name="pb", bufs=1))
        ht = [mktile(pb, [P, L], f"h{ct}") for ct in range(CT)]

        # ---- GroupNorm ----------------------------------------------
        with ExitStack() as ph:
            px = ph.enter_context(tc.tile_pool(name="px", bufs=1))
            pgs = ph.enter_context(tc.tile_pool(name="pgs", bufs=1))
            pgp = ph.enter_context(tc.tile_pool(name="pgp", bufs=1, space="PSUM"))
            xs = []
            for ct in range(CT):
                t = mktile(px, [P, L], f"x{ct}")
                for lc in range(LC):
                    sl = slice(lc * 512, (lc + 1) * 512)
                    nc.sync.dma_start(out=r(t[:, sl]),
                                      in_=r(x_ext[ct * P:(ct + 1) * P, sl]))
                xs.append(t)
            xsq = [mktile(px, [P, L], f"xsq{ct}") for ct in range(CT)]
            for ct in range(CT):
                for lc in range(LC):
                    sl = slice(lc * 512, (lc + 1) * 512)
                    nc.vector.tensor_mul(r(xsq[ct][:, sl]),
                                         xs[ct][:, sl], xs[ct][:, sl])
            ps_s = mktile(pgp, [G, 512], "ps_s")
            ps_q = mktile(pgp, [G, 512], "ps_q")
            n = 0
            for ct in range(CT):
                for lc in range(LC):
                    sl = slice(lc * 512, (lc + 1) * 512)
                    st, sp = (n == 0), (n == CT * LC - 1)
                    nc.tensor.matmul(ps_s[:], r(indt[ct][:]), r(xs[ct][:, sl]),
                                     start=st, stop=sp)
                    nc.tensor.matmul(ps_q[:], r(indt[ct][:]), r(xsq[ct][:, sl]),
                                     start=st, stop=sp)
                    n += 1
            sums = mktile(pgs, [G, 1], "sums")
            sumq = mktile(pgs, [G, 1], "sumq")
            nc.vector.reduce_sum(sums[:], ps_s[:], axis=AX.X)
            nc.vector.reduce_sum(sumq[:], ps_q[:], axis=AX.X)
            inv_n = 1.0 / float((C // G) * L)
            mrs = mktile(pgs, [G, 2], "mrs")      # col0 = mean, col1 = rstd
            var = mktile(pgs, [G, 1], "var")
            sqv = mktile(pgs, [G, 1], "sqv")
            nc.vector.tensor_scalar_mul(mrs[:, 0:1], sums[:], inv_n)
            nc.vector.tensor_scalar_mul(var[:], sumq[:], inv_n)
            nc.vector.tensor_mul(sqv[:], mrs[:, 0:1], mrs[:, 0:1])
            nc.vector.tensor_sub(var[:], var[:], sqv[:])
            eps_t = mktile(pgs, [G, 1], "eps")
            nc.vector.memset(eps_t[:], EPS)
            nc.scalar.activation(sqv[:], var[:], ACT.Ln, bias=eps_t[:])
            nc.scalar.activation(mrs[:, 1:2], sqv[:], ACT.Exp, scale=-0.5)
            # broadcast group stats to channels via PE: bc[c, :] = mrs[g(c), :]
            for ct in range(CT):
                bc_ps = mktile(pgp, [P, 2], f"bcps{ct}")
                nc.tensor.matmul(bc_ps[:], indTt[:, ct * P:(ct + 1) * P],
                                 mrs[:], start=True, stop=True)
                s_t = mktile(pgs, [P, 1], f"s{ct}")
                t_t = mktile(pgs, [P, 1], f"t{ct}")
                nc.vector.tensor_mul(s_t[:], bc_ps[:, 1:2], bias["gnw"][ct][:])
                nc.vector.tensor_mul(t_t[:], bc_ps[:, 0:1], s_t[:])
                nc.vector.tensor_sub(t_t[:], bias["gnb"][ct][:], t_t[:])
                nc.vector.tensor_scalar(r(ht[ct][:]), xs[ct][:], s_t[:], t_t[:],
                                        op0=OP.mult, op1=OP.add)

        # ---- pool C: q/k/VT (live through attention) -----------------
        pc = top.enter_context(tc.tile_pool(name="pc", bufs=1))
        qt = [mktile(pc, [P, DH], f"q{p}", dtype=f16) for p in range(CT)]
        kt = [mktile(pc, [P, L], f"k{p}", dtype=f16) for p in range(CT)]
        vt = [mktile(pc, [P, 65 * ECH], f"vt{h}", dtype=f16) for h in range(NH)]

        # ---- QKV + V^T, as chunk closures ---------------------------
        # Pair 0's chunks are emitted up front; pair 1's are deferred into
        # pair 0's first attention block so they execute under the
        # ScalarE-bound exp stream. All QKV/transpose PSUM goes through the
        # shared "aux" pool (also used by the softmax-denominator broadcast
        # and the projection).
        vpair = [mktile(pc, [P, L], f"v{p}") for p in range(CT)]

        def qkv_chunks(p, paux):
            work = []
            osl = slice(p * P, (p + 1) * P)
            streams = [("wqt", "bq", qt[p], DBLK),
                       ("wkt", "bk", kt[p], LC),
                       ("wvt", "bv", vpair[p], LC)]
            for wnm, bnm, dst, nchunks in streams:
                for cchunk in range(nchunks):
                    def go(wnm=wnm, bnm=bnm, dst=dst, cchunk=cchunk):
                        sl = slice(cchunk * 512, (cchunk + 1) * 512)
                        ps = paux.tile([P, 512], f32, name="aux", tag="aux")
                        for ct in range(CT):
                            nc.tensor.matmul(ps[:], r(wts[wnm][ct][:, osl]),
                                             r(ht[ct][:, sl]),
                                             start=(ct == 0), stop=(ct == CT - 1))
                        dst_ap = dst[:, sl]
                        if dst_ap.dtype == f32:
                            dst_ap = r(dst_ap)
                        nc.vector.tensor_scalar_add(dst_ap, ps[:], bias[bnm][p][:])
                    work.append(go)
            for ecg in range(ECH // 4):
                def go(p=p, ecg=ecg):
                    pst = paux.tile([P, 512], f32, name="aux", tag="aux")
                    for j in range(4):
                        esl = slice((ecg * 4 + j) * P, (ecg * 4 + j + 1) * P)
                        nc.tensor.transpose(pst[:, j * P:(j + 1) * P],
                                            vpair[p][:, esl], ident[:])
                    for h01 in range(2):
                        head = 2 * p + h01
                        outap = (vt[head][:, ecg * 260:(ecg + 1) * 260]
                                 .rearrange("p (a b) -> p a b", b=65)[:, :, 0:64])
                        inap = (pst[:].rearrange("p (a b) -> p a b", b=P)
                                [:, :, h01 * 64:(h01 + 1) * 64])
                        nc.vector.tensor_copy(out=outap, in_=inap)
                work.append(go)
            for h01 in range(2):
                def go(p=p, h01=h01):
                    h = 2 * p + h01
                    ones_ap = (vt[h][:].rearrange("p (a b) -> p a b", b=65)
                               [:, :, 64:65])
                    nc.vector.tensor_copy(
                        out=ones_ap,
                        in_=ones_src[:, 0:ECH].rearrange("p a -> p a ()"))
                work.append(go)
            return work

        # ---- attention + per-db projection --------------------------
        # db is the outer loop so that projection for a finished d-block
        # overlaps the next d-block's attention; epilogues (softmax
        # normalize) are emitted deferred, a few ec-iterations into the
        # following (p, db) chunk, so their latency never stalls the PE
        # stream at chunk boundaries.
        with ExitStack() as ph:
            pe_s = ph.enter_context(tc.tile_pool(name="pes", bufs=3))
            pf = ph.enter_context(tc.tile_pool(name="pf", bufs=1))
            pfs = ph.enter_context(tc.tile_pool(name="pfs", bufs=3))
            psp = ph.enter_context(tc.tile_pool(name="psp", bufs=2, space="PSUM"))
            pap = ph.enter_context(tc.tile_pool(name="pap", bufs=2, space="PSUM"))
            paux = ph.enter_context(tc.tile_pool(name="paux", bufs=2, space="PSUM"))
            xres = []
            for ct in range(CT):
                t = mktile(pf, [P, DH], f"xr{ct}")
                nc.sync.dma_start(out=t[:], in_=x_ext[ct * P:(ct + 1) * P, 0:DH])
                xres.append(t)
            p0_work = qkv_chunks(0, paux)
            # early: q (DBLK) + k (LC) chunks, plus v chunk 0, vT group 0 and
            # the two ones-columns -- exactly what S[ec0]/PV[ec0] need.
            # late: remaining v chunks interleaved with their vT groups so
            # group g is always emitted well before PV consumes it at ec=4g.
            n_qk = DBLK + LC
            early = (p0_work[:n_qk] + [p0_work[n_qk]]
                     + [p0_work[n_qk + LC]] + p0_work[-2:])
            late = []
            for g in range(1, LC):
                late.append(p0_work[n_qk + g])           # v chunk g
                late.append(p0_work[n_qk + LC + g])      # vT group g
            for fn in early:
                fn()
            deferred = late + qkv_chunks(1, paux)

            def epilogue_head(p, db, acc, h01):
                dsl = slice(db * 512, (db + 1) * 512)
                head = 2 * p + h01
                zc = mktile(pe_s, [65, 512], "zc")
                nc.vector.tensor_copy(out=r(zc[:]), in_=acc[h01][:])
                zb_ps = paux.tile([P, 512], f32, name="aux", tag="aux")
                nc.tensor.matmul(zb_ps[0:64, :], r(ones_src[64:65, 0:64]),
                                 r(zc[64:65, :]), start=True, stop=True)
                zb = mktile(pe_s, [64, 512], "zb")
                with nc.allow_low_precision("softmax denom, well conditioned"):
                    nc.vector.reciprocal_approx_fast(zb[:], zb_ps[0:64, :])
                nc.vector.tensor_mul(r(ao[head][:, dsl]),
                                     zc[0:64, :], zb[:])

            def proj_ot(db, ot):
                dsl = slice(db * 512, (db + 1) * 512)
                osl = slice(ot * P, (ot + 1) * P)
                ps = paux.tile([P, 512], f32, name="aux", tag="aux")
                for h in range(NH):
                    nc.tensor.matmul(ps[:], r(wpth[h][:, osl]),
                                     r(ao[h][:, dsl]),
                                     start=(h == 0), stop=(h == NH - 1))
                osb = mktile(pfs, [P, 512], "osb")
                nc.vector.scalar_tensor_tensor(
                    out=osb[:], in0=ps[:], scalar=bias["bp"][ot][:],
                    in1=xres[ot][:, dsl], op0=OP.add, op1=OP.add)
                nc.sync.dma_start(out=out_ext[osl, dsl], in_=osb[:])

            pending = []
            epi_unit = []
            for db in range(DBLK):
                for p in range(CT):
                    dsl = slice(db * 512, (db + 1) * 512)
                    acc = [mktile(pap, [65, 512], "acc") for _ in range(2)]
                    for ec in range(ECH):
                        esl = slice(ec * P, (ec + 1) * P)
                        sps = mktile(psp, [P, 1024], "sps")
                        for h01 in range(2):
                            hsl = slice(h01 * 64, (h01 + 1) * 64)
                            nc.tensor.matmul(sps[:, h01 * 512:(h01 + 1) * 512],
                                             kt[p][hsl, esl], qt[p][hsl, dsl],
                                             start=True, stop=True)
                        pt = mktile(pe_s, [P, 1024], "pt", dtype=f16)
                        nc.scalar.activation(pt[:], sps[:], ACT.Exp, scale=SCALE)
                        for h01 in range(2):
                            nc.tensor.matmul(
                                acc[h01][:],
                                vt[2 * p + h01][:, ec * 65:(ec + 1) * 65],
                                pt[:, h01 * 512:(h01 + 1) * 512],
                                start=(ec == 0), stop=(ec == ECH - 1))
                        if ec >= 2:
                            for _ in range(2):
                                if deferred:
                                    deferred.pop(0)()
                                elif pending:
                                    pending.pop(0)()
                                    break
                    for h01 in range(2):
                        pending.append(
                            lambda p=p, db=db, acc=acc, h01=h01:
                            epilogue_head(p, db, acc, h01))
                    if p == CT - 1:
                        for ot in range(CT):
                            pending.append(lambda db=db, ot=ot: proj_ot(db, ot))
            for fn in deferred + pending:
                fn()

    nc.compile()
    return nc


def _in_maps(inputs):
    x = np.asarray(inputs["x"], dtype=np.float32)
    gnw = np.ascontiguousarray(np.asarray(inputs["gn_w"], np.float32).reshape(C, 1))
    gnb = np.ascontiguousarray(np.asarray(inputs["gn_b"], np.float32).reshape(C, 1))
    wqt = np.ascontiguousarray(np.asarray(inputs["wq"], np.float32).T)
    wkt = np.ascontiguousarray(np.asarray(inputs["wk"], np.float32).T)
    wvt = np.ascontiguousarray(np.asarray(inputs["wv"], np.float32).T)
    wpt = np.ascontiguousarray(np.asarray(inputs["wp"], np.float32).T)
    bq = np.ascontiguousarray(np.asarray(inputs["bq"], np.float32).reshape(C, 1))
    bk = np.ascontiguousarray(np.asarray(inputs["bk"], np.float32).reshape(C, 1))
    bv = np.ascontiguousarray(np.asarray(inputs["bv"], np.float32).reshape(C, 1))
    bp = np.ascontiguousarray(np.asarray(inputs["bp"], np.float32).reshape(C, 1))
    ind = np.zeros((C, G), np.float32)
    ind[np.arange(C), np.arange(C) // (C // G)] = 1.0
    indT = np.ascontiguousarray(ind.T)
    ones = np.ones((P, 64), np.float32)
    common = dict(wqt=wqt, wkt=wkt, wvt=wvt, wpt=wpt, bq=bq, bk=bk, bv=bv,
                  bp=bp, gnw=gnw, gnb=gnb, ind=ind, indT=indT, ones=ones)
    maps = []
    for core in range(NCORES):
        b, half = core // 2, core % 2
        xb = np.ascontiguousarray(x[b].reshape(C, L))
        if half == 1:
            xb = np.ascontiguousarray(
                np.concatenate([xb[:, DH:], xb[:, :DH]], axis=1))
        maps.append(dict(common, x=xb))
    return maps


def kernel(**inputs) -> np.ndarray:
    from concourse.bass_utils import run_bass_kernel_spmd

    if "nc" not in _CACHE:
        _CACHE["nc"] = _build_nc()
    nc = _CACHE["nc"]
    res = run_bass_kernel_spmd(nc, _in_maps(inputs), core_ids=list(range(NCORES)))
    out = np.empty((B, C, L), np.float32)
    for core in range(NCORES):
        b, half = core // 2, core % 2
        out[b][:, half * DH:(half + 1) * DH] = res.results[core]["out"]
    return out.reshape(B, C, H, W)


# revision 26
# speedup vs baseline: 1.0201x; 1.0201x over previous
"""Trainium2 Bass kernel for an AttentionBlock:
GroupNorm(8 groups) -> 1x1 conv q/k/v -> multi-head attention (4 heads)
-> 1x1 conv proj -> residual add.

Shapes (hardcoded): x [4, 256, 64, 64]; L = 64*64 = 4096; head dim 64.

Sharding: 8 cores = (batch, query-half). Each core computes the full
GroupNorm + K/V for its batch, and attention + projection + residual for
its half (2048) of the query positions. Host permutes each batch's pixel
columns so a core's query half is always columns 0:2048 (attention is
permutation-invariant over key positions, GroupNorm over pixels), so all
8 cores run one SPMD program. No collectives; host just concatenates.

Key kernel ideas:
- S is computed transposed ([key e x query d] via lhsT=K-chunk, rhs=Q)
  so softmax needs no cross-partition reductions.
- Softmax skips the max-subtraction (logits are within [-8, 8] for
  normalized inputs; exp is exact to ~2 ULP there) -> exp(S^T) directly
  on ScalarE out of PSUM, scale=hd^-0.5 folded into the activation.
- The softmax denominator Z is produced by the PV matmul itself: V^T is
  materialized per head with a 65th column of ones, so PV output row 64
  accumulates sum_e(P) while rows 0..63 accumulate V@P.
- Attention operands (q, k, P=exp(S), V^T) are fp16 (~5e-4 quantization):
  fp16 streams 1 column/cycle on the normal PE datapath, which keeps the
  HAM clock gate warm at 2.4 GHz -- float32r runs on the transpose-mode
  path that never warms HAM, capping PE at ~1.2 GHz. Non-attention
  matmuls (GroupNorm stats, QKV, projection) stay float32r, accumulation
  is always fp32 in PSUM.
- The two heads of a pair are packed into the PE array with row tiling
  (K=64 each at rows 0-63/64-127); db is the outer loop so projection
  overlaps the next block's attention; pair-1 QKV/V^T and all softmax
  epilogues are emitted deferred so they execute under the ScalarE-bound
  exp stream (ScalarE exp of all 33.5M logits/core is the ~275us floor).
"""

import numpy as np

B, C, H, W = 4, 256, 64, 64
NH, G, EPS = 4, 8, 1e-5
L = H * W            # 4096
DH = L // 2          # query positions per core
HD = C // NH         # 64
P = 128              # SBUF partitions
CT = C // P          # channel tiles (2)
LC = L // 512        # 8 key-dim 512-chunks
DBLK = DH // 512     # 4 query-dim 512-blocks
ECH = L // P         # 32 key-dim 128-chunks
SCALE = float(HD) ** -0.5
NCORES = 8

_CACHE = {}


def _build_nc():
    import concourse.bacc as bacc
    import concourse.bass as bass
    import concourse.mybir as mybir
    import concourse.tile as tile
    from concourse.masks import make_identity
    from contextlib import ExitStack

    f32 = mybir.dt.float32
    f16 = mybir.dt.float16
    f32r = mybir.dt.float32r
    AX = mybir.AxisListType
    OP = mybir.AluOpType
    ACT = mybir.ActivationFunctionType

    def r(ap):
        return ap.bitcast(f32r)

    def mktile(pool, shape, tag, dtype=None):
        return pool.tile(shape, dtype or f32, name=tag, tag=tag)

    nc = bacc.Bacc(trn_type="TRN2", target_bir_lowering=False, num_devices=NCORES)

    x_ext = nc.declare_dram_parameter("x", [C, L], f32, isOutput=False)
    wq_ext = nc.declare_dram_parameter("wqt", [C, C], f32, isOutput=False)
    wk_ext = nc.declare_dram_parameter("wkt", [C, C], f32, isOutput=False)
    wv_ext = nc.declare_dram_parameter("wvt", [C, C], f32, isOutput=False)
    wp_ext = nc.declare_dram_parameter("wpt", [C, C], f32, isOutput=False)
    bq_ext = nc.declare_dram_parameter("bq", [C, 1], f32, isOutput=False)
    bk_ext = nc.declare_dram_parameter("bk", [C, 1], f32, isOutput=False)
    bv_ext = nc.declare_dram_parameter("bv", [C, 1], f32, isOutput=False)
    bp_ext = nc.declare_dram_parameter("bp", [C, 1], f32, isOutput=False)
    gnw_ext = nc.declare_dram_parameter("gnw", [C, 1], f32, isOutput=False)
    gnb_ext = nc.declare_dram_parameter("gnb", [C, 1], f32, isOutput=False)
    ones_ext = nc.declare_dram_parameter("ones", [P, 64], f32, isOutput=False)
    ind_ext = nc.declare_dram_parameter("ind", [C, G], f32, isOutput=False)
    indT_ext = nc.declare_dram_parameter("indT", [G, C], f32, isOutput=False)
    out_ext = nc.declare_dram_parameter("out", [C, DH], f32, isOutput=True)

    with tile.TileContext(nc) as tc, ExitStack() as top:
        # ---- pool A: kernel-long tiles -------------------------------
        pa = top.enter_context(tc.tile_pool(name="pa", bufs=1))
        wts = {}
        for nm, ext in (("wqt", wq_ext), ("wkt", wk_ext), ("wvt", wv_ext)):
            wts[nm] = []
            for ct in range(CT):
                t = mktile(pa, [P, C], f"{nm}{ct}")
                nc.gpsimd.dma_start(out=r(t[:]), in_=r(ext[ct * P:(ct + 1) * P, :]))
                wts[nm].append(t)
        # proj weight per head (base partition 0 for K=64 contraction)
        wpth = []
        for h in range(NH):
            t = mktile(pa, [HD, C], f"wpt{h}")
            nc.gpsimd.dma_start(out=r(t[:]), in_=r(wp_ext[h * HD:(h + 1) * HD, :]))
            wpth.append(t)
        bias = {}
        for nm, ext in (("bq", bq_ext), ("bk", bk_ext), ("bv", bv_ext),
                        ("bp", bp_ext), ("gnw", gnw_ext), ("gnb", gnb_ext)):
            bias[nm] = []
            for ct in range(CT):
                t = mktile(pa, [P, 1], f"{nm}{ct}")
                nc.sync.dma_start(out=t[:], in_=ext[ct * P:(ct + 1) * P, :])
                bias[nm].append(t)
        indt = []
        for ct in range(CT):
            t = mktile(pa, [P, G], f"ind{ct}")
            nc.sync.dma_start(out=r(t[:]), in_=r(ind_ext[ct * P:(ct + 1) * P, :]))
            indt.append(t)
        indTt = mktile(pa, [G, C], "indTt")
        nc.sync.dma_start(out=indTt[:], in_=indT_ext[:])
        ident = mktile(pa, [P, P], "ident")
        make_identity(nc, ident[:])
        ones_src = mktile(pa, [P, 64], "ones_src")
        nc.sync.dma_start(out=r(ones_src[:]), in_=r(ones_ext[:]))
        # attention output, one tile per head [64, DH] at base partition 0
        ao = [mktile(pa, [HD, DH], f"ao{h}") for h in range(NH)]

        # ---- pool B: normalized activations h ------------------------
        pb = top.enter_context(tc.tile_pool(name="pb", bufs=1))
        ht = [mktile(pb, [P, L], f"h{ct}") for ct in range(CT)]

        # ---- GroupNorm ----------------------------------------------
        with ExitStack() as ph:
            px = ph.enter_context(tc.tile_pool(name="px", bufs=1))
            pgs = ph.enter_context(tc.tile_pool(name="pgs", bufs=1))
            pgp = ph.enter_context(tc.tile_pool(name="pgp", bufs=1, space="PSUM"))
            xs = []
            for ct in range(CT):
                t = mktile(px, [P, L], f"x{ct}")
                for lc in range(LC):
                    sl = slice(lc * 512, (lc + 1) * 512)
                    nc.sync.dma_start(out=r(t[:, sl]),
                                      in_=r(x_ext[ct * P:(ct + 1) * P, sl]))
                xs.append(t)
            xsq = [mktile(px, [P, L], f"xsq{ct}") for ct in range(CT)]
            for ct in range(CT):
                for lc in range(LC):
                    sl = slice(lc * 512, (lc + 1) * 512)
                    nc.vector.tensor_mul(r(xsq[ct][:, sl]),
                                         xs[ct][:, sl], xs[ct][:, sl])
            ps_s = mktile(pgp, [G, 512], "ps_s")
            ps_q = mktile(pgp, [G, 512], "ps_q")
            n = 0
            for ct in range(CT):
                for lc in range(LC):
                    sl = slice(lc * 512, (lc + 1) * 512)
                    st, sp = (n == 0), (n == CT * LC - 1)
                    nc.tensor.matmul(ps_s[:], r(indt[ct][:]), r(xs[ct][:, sl]),
                                     start=st, stop=sp)
                    nc.tensor.matmul(ps_q[:], r(indt[ct][:]), r(xsq[ct][:, sl]),
                                     start=st, stop=sp)
                    n += 1
            sums = mktile(pgs, [G, 1], "sums")
            sumq = mktile(pgs, [G, 1], "sumq")
            nc.vector.reduce_sum(sums[:], ps_s[:], axis=AX.X)
            nc.vector.reduce_sum(sumq[:], ps_q[:], axis=AX.X)
            inv_n = 1.0 / float((C // G) * L)
            mrs = mktile(pgs, [G, 2], "mrs")      # col0 = mean, col1 = rstd
            var = mktile(pgs, [G, 1], "var")
            sqv = mktile(pgs, [G, 1], "sqv")
            nc.vector.tensor_scalar_mul(mrs[:, 0:1], sums[:], inv_n)
            nc.vector.tensor_scalar_mul(var[:], sumq[:], inv_n)
            nc.vector.tensor_mul(sqv[:], mrs[:, 0:1], mrs[:, 0:1])
            nc.vector.tensor_sub(var[:], var[:], sqv[:])
            eps_t = mktile(pgs, [G, 1], "eps")
            nc.vector.memset(eps_t[:], EPS)
            nc.scalar.activation(sqv[:], var[:], ACT.Ln, bias=eps_t[:])
            nc.scalar.activation(mrs[:, 1:2], sqv[:], ACT.Exp, scale=-0.5)
            # broadcast group stats to channels via PE: bc[c, :] = mrs[g(c), :]
            for ct in range(CT):
                bc_ps = mktile(pgp, [P, 2], f"bcps{ct}")
                nc.tensor.matmul(bc_ps[:], indTt[:, ct * P:(ct + 1) * P],
                                 mrs[:], start=True, stop=True)
                s_t = mktile(pgs, [P, 1], f"s{ct}")
                t_t = mktile(pgs, [P, 1], f"t{ct}")
                nc.vector.tensor_mul(s_t[:], bc_ps[:, 1:2], bias["gnw"][ct][:])
                nc.vector.tensor_mul(t_t[:], bc_ps[:, 0:1], s_t[:])
                nc.vector.tensor_sub(t_t[:], bias["gnb"][ct][:], t_t[:])
                nc.vector.tensor_scalar(r(ht[ct][:]), xs[ct][:], s_t[:], t_t[:],
                                        op0=OP.mult, op1=OP.add)

        # ---- pool C: q/k/VT (live through attention) -----------------
        pc = top.enter_context(tc.tile_pool(name="pc", bufs=1))
        qt = [mktile(pc, [P, DH], f"q{p}", dtype=f16) for p in range(CT)]
        kt = [mktile(pc, [P, L], f"k{p}", dtype=f16) for p in range(CT)]
        vt = [mktile(pc, [P, 65 * ECH], f"vt{h}", dtype=f16) for h in range(NH)]

        # ---- QKV + V^T, as chunk closures ---------------------------
        # Pair 0's chunks are emitted up front; pair 1's are deferred into
        # pair 0's first attention block so they execute under the
        # ScalarE-bound exp stream. All QKV/transpose PSUM goes through the
        # shared "aux" pool (also used by the softmax-denominator broadcast
        # and the projection).
        vpair = [mktile(pc, [P, L], f"v{p}") for p in range(CT)]

        def qkv_chunks(p, paux):
            work = []
            osl = slice(p * P, (p + 1) * P)
            streams = [("wqt", "bq", qt[p], DBLK),
                       ("wkt", "bk", kt[p], LC),
                       ("wvt", "bv", vpair[p], LC)]
            for wnm, bnm, dst, nchunks in streams:
                for cchunk in range(nchunks):
                    def go(wnm=wnm, bnm=bnm, dst=dst, cchunk=cchunk):
                        sl = slice(cchunk * 512, (cchunk + 1) * 512)
                        ps = paux.tile([P, 512], f32, name="aux", tag="aux")
                        for ct in range(CT):
                            nc.tensor.matmul(ps[:], r(wts[wnm][ct][:, osl]),
                                             r(ht[ct][:, sl]),
                                             start=(ct == 0), stop=(ct == CT - 1))
                        dst_ap = dst[:, sl]
                        if dst_ap.dtype == f32:
                            dst_ap = r(dst_ap)
                        nc.vector.tensor_scalar_add(dst_ap, ps[:], bias[bnm][p][:])
                    work.append(go)
            for ecg in range(ECH // 4):
                def go(p=p, ecg=ecg):
                    pst = paux.tile([P, 512], f32, name="aux", tag="aux")
                    for j in range(4):
                        esl = slice((ecg * 4 + j) * P, (ecg * 4 + j + 1) * P)
                        nc.tensor.transpose(pst[:, j * P:(j + 1) * P],
                                            vpair[p][:, esl], ident[:])
                    for h01 in range(2):
                        head = 2 * p + h01
                        outap = (vt[head][:, ecg * 260:(ecg + 1) * 260]
                                 .rearrange("p (a b) -> p a b", b=65)[:, :, 0:64])
                        inap = (pst[:].rearrange("p (a b) -> p a b", b=P)
                                [:, :, h01 * 64:(h01 + 1) * 64])
                        nc.vector.tensor_copy(out=outap, in_=inap)
                work.append(go)
            for h01 in range(2):
                def go(p=p, h01=h01):
                    h = 2 * p + h01
                    ones_ap = (vt[h][:].rearrange("p (a b) -> p a b", b=65)
                               [:, :, 64:65])
                    nc.vector.tensor_copy(
                        out=ones_ap,
                        in_=ones_src[:, 0:ECH].rearrange("p a -> p a ()"))
                work.append(go)
            return work

        # ---- attention + per-db projection --------------------------
        # db is the outer loop so that projection for a finished d-block
        # overlaps the next d-block's attention; epilogues (softmax
        # normalize) are emitted deferred, a few ec-iterations into the
        # following (p, db) chunk, so their latency never stalls the PE
        # stream at chunk boundaries.
        with ExitStack() as ph:
            pe_s = ph.enter_context(tc.tile_pool(name="pes", bufs=3))
            pf = ph.enter_context(tc.tile_pool(name="pf", bufs=1))
            pfs = ph.enter_context(tc.tile_pool(name="pfs", bufs=3))
            psp = ph.enter_context(tc.tile_pool(name="psp", bufs=2, space="PSUM"))
            pap = ph.enter_context(tc.tile_pool(name="pap", bufs=2, space="PSUM"))
            paux = ph.enter_context(tc.tile_pool(name="paux", bufs=2, space="PSUM"))
            xres = []
            for ct in range(CT):
                t = mktile(pf, [P, DH], f"xr{ct}")
                nc.sync.dma_start(out=t[:], in_=x_ext[ct * P:(ct + 1) * P, 0:DH])
                xres.append(t)
            p0_work = qkv_chunks(0, paux)
            # early: q (DBLK) + k (LC) chunks, plus v chunk 0, vT group 0 and
            # the two ones-columns -- exactly what S[ec0]/PV[ec0] need.
            # late: remaining v chunks interleaved with their vT groups so
            # group g is always emitted well before PV consumes it at ec=4g.
            n_qk = DBLK + LC
            early = (p0_work[:n_qk] + [p0_work[n_qk]]
                     + [p0_work[n_qk + LC]] + p0_work[-2:])
            late = []
            for g in range(1, LC):
                late.append(p0_work[n_qk + g])           # v chunk g
                late.append(p0_work[n_qk + LC + g])      # vT group g
            for fn in early:
                fn()
            deferred = late + qkv_chunks(1, paux)

            def epilogue_head(p, db, acc, h01):
                dsl = slice(db * 512, (db + 1) * 512)
                head = 2 * p + h01
                zc = mktile(pe_s, [65, 512], "zc")
                nc.vector.tensor_copy(out=r(zc[:]), in_=acc[h01][:])
                zb_ps = paux.tile([P, 512], f32, name="aux", tag="aux")
                nc.tensor.matmul(zb_ps[0:64, :], r(ones_src[64:65, 0:64]),
                                 r(zc[64:65, :]), start=True, stop=True)
                zb = mktile(pe_s, [64, 512], "zb")
                with nc.allow_low_precision("softmax denom, well conditioned"):
                    nc.vector.reciprocal_approx_fast(zb[:], zb_ps[0:64, :])
                nc.vector.tensor_mul(r(ao[head][:, dsl]),
                                     zc[0:64, :], zb[:])

            def proj_ot(db, ot):
                dsl = slice(db * 512, (db + 1) * 512)
                osl = slice(ot * P, (ot + 1) * P)
                ps = paux.tile([P, 512], f32, name="aux", tag="aux")
                for h in range(NH):
                    nc.tensor.matmul(ps[:], r(wpth[h][:, osl]),
                                     r(ao[h][:, dsl]),
                                     start=(h == 0), stop=(h == NH - 1))
                osb = mktile(pfs, [P, 512], "osb")
                nc.vector.scalar_tensor_tensor(
                    out=osb[:], in0=ps[:], scalar=bias["bp"][ot][:],
                    in1=xres[ot][:, dsl], op0=OP.add, op1=OP.add)
                nc.sync.dma_start(out=out_ext[osl, dsl], in_=osb[:])

            pending = []
            epi_unit = []
            for db in range(DBLK):
                for p in range(CT):
                    dsl = slice(db * 512, (db + 1) * 512)
                    acc = [mktile(pap, [65, 512], "acc") for _ in range(2)]
                    for ec in range(ECH):
                        esl = slice(ec * P, (ec + 1) * P)
                        sps = mktile(psp, [P, 1024], "sps")
                        for h01 in range(2):
                            hsl = slice(h01 * 64, (h01 + 1) * 64)
                            nc.tensor.matmul(sps[:, h01 * 512:(h01 + 1) * 512],
                                             kt[p][hsl, esl], qt[p][hsl, dsl],
                                             start=True, stop=True)
                        pt = mktile(pe_s, [P, 1024], "pt", dtype=f16)
                        nc.scalar.activation(pt[:], sps[:], ACT.Exp, scale=SCALE)
                        for h01 in range(2):
                            nc.tensor.matmul(
                                acc[h01][:],
                                vt[2 * p + h01][:, ec * 65:(ec + 1) * 65],
                                pt[:, h01 * 512:(h01 + 1) * 512],
                                start=(ec == 0), stop=(ec == ECH - 1))
                        if ec >= 2:
                            for _ in range(2):
                                if deferred:
                                    deferred.pop(0)()
                                elif pending:
                                    pending.pop(0)()
                                    break
                    for h01 in range(2):
                        pending.append(
                            lambda p=p, db=db, acc=acc, h01=h01:
                            epilogue_head(p, db, acc, h01))
                    if p == CT - 1:
                        for ot in range(CT):
                            pending.append(lambda db=db, ot=ot: proj_ot(db, ot))
            for fn in deferred + pending:
                fn()

    nc.compile()
    return nc


def _in_maps(inputs):
    x = np.asarray(inputs["x"], dtype=np.float32)
    gnw = np.ascontiguousarray(np.asarray(inputs["gn_w"], np.float32).reshape(C, 1))
    gnb = np.ascontiguousarray(np.asarray(inputs["gn_b"], np.float32).reshape(C, 1))
    wqt = np.ascontiguousarray(np.asarray(inputs["wq"], np.float32).T)
    wkt = np.ascontiguousarray(np.asarray(inputs["wk"], np.float32).T)
    wvt = np.ascontiguousarray(np.asarray(inputs["wv"], np.float32).T)
    wpt = np.ascontiguousarray(np.asarray(inputs["wp"], np.float32).T)
    bq = np.ascontiguousarray(np.asarray(inputs["bq"], np.float32).reshape(C, 1))
    bk = np.ascontiguousarray(np.asarray(inputs["bk"], np.float32).reshape(C, 1))
    bv = np.ascontiguousarray(np.asarray(inputs["bv"], np.float32).reshape(C, 1))
    bp = np.ascontiguousarray(np.asarray(inputs["bp"], np.float32).reshape(C, 1))
    ind = np.zeros((C, G), np.float32)
    ind[np.arange(C), np.arange(C) // (C // G)] = 1.0
    indT = np.ascontiguousarray(ind.T)
    ones = np.ones((P, 64), np.float32)
    common = dict(wqt=wqt, wkt=wkt, wvt=wvt, wpt=wpt, bq=bq, bk=bk, bv=bv,
                  bp=bp, gnw=gnw, gnb=gnb, ind=ind, indT=indT, ones=ones)
    maps = []
    for core in range(NCORES):
        b, half = core // 2, core % 2
        xb = np.ascontiguousarray(x[b].reshape(C, L))
        if half == 1:
            xb = np.ascontiguousarray(
                np.concatenate([xb[:, DH:], xb[:, :DH]], axis=1))
        maps.append(dict(common, x=xb))
    return maps


def kernel(**inputs) -> np.ndarray:
    from concourse.bass_utils import run_bass_kernel_spmd

    if "nc" not in _CACHE:
        _CACHE["nc"] = _build_nc()
    nc = _CACHE["nc"]
    res = run_bass_kernel_spmd(nc, _in_maps(inputs), core_ids=list(range(NCORES)))
    out = np.empty((B, C, L), np.float32)
    for core in range(NCORES):
        b, half = core // 2, core % 2
        out[b][:, half * DH:(half + 1) * DH] = res.results[core]["out"]
    return out.reshape(B, C, H, W)


# revision 27
# speedup vs baseline: 1.0301x; 1.0098x over previous
"""Trainium2 Bass kernel for an AttentionBlock:
GroupNorm(8 groups) -> 1x1 conv q/k/v -> multi-head attention (4 heads)
-> 1x1 conv proj -> residual add.

Shapes (hardcoded): x [4, 256, 64, 64]; L = 64*64 = 4096; head dim 64.

Sharding: 8 cores = (batch, query-half). Each core computes the full
GroupNorm + K/V for its batch, and attention + projection + residual for
its half (2048) of the query positions. Host permutes each batch's pixel
columns so a core's query half is always columns 0:2048 (attention is
permutation-invariant over key positions, GroupNorm over pixels), so all
8 cores run one SPMD program. No collectives; host just concatenates.

Key kernel ideas:
- S is computed transposed ([key e x query d] via lhsT=K-chunk, rhs=Q)
  so softmax needs no cross-partition reductions.
- Softmax skips the max-subtraction (logits are within [-8, 8] for
  normalized inputs; exp is exact to ~2 ULP there) -> exp(S^T) directly
  on ScalarE out of PSUM, scale=hd^-0.5 folded into the activation.
- The softmax denominator Z is produced by the PV matmul itself: V^T is
  materialized per head with a 65th column of ones, so PV output row 64
  accumulates sum_e(P) while rows 0..63 accumulate V@P.
- Attention operands (q, k, P=exp(S), V^T) are fp16 (~5e-4 quantization):
  fp16 streams 1 column/cycle on the normal PE datapath, which keeps the
  HAM clock gate warm at 2.4 GHz -- float32r runs on the transpose-mode
  path that never warms HAM, capping PE at ~1.2 GHz. Non-attention
  matmuls (GroupNorm stats, QKV, projection) stay float32r, accumulation
  is always fp32 in PSUM.
- The two heads of a pair are packed into the PE array with row tiling
  (K=64 each at rows 0-63/64-127); db is the outer loop so projection
  overlaps the next block's attention; pair-1 QKV/V^T and all softmax
  epilogues are emitted deferred so they execute under the ScalarE-bound
  exp stream (ScalarE exp of all 33.5M logits/core is the ~275us floor).
"""

import numpy as np

B, C, H, W = 4, 256, 64, 64
NH, G, EPS = 4, 8, 1e-5
L = H * W            # 4096
DH = L // 2          # query positions per core
HD = C // NH         # 64
P = 128              # SBUF partitions
CT = C // P          # channel tiles (2)
LC = L // 512        # 8 key-dim 512-chunks
DBLK = DH // 512     # 4 query-dim 512-blocks
ECH = L // P         # 32 key-dim 128-chunks
SCALE = float(HD) ** -0.5
NCORES = 8

_CACHE = {}


def _build_nc():
    import concourse.bacc as bacc
    import concourse.bass as bass
    import concourse.mybir as mybir
    import concourse.tile as tile
    from concourse.masks import make_identity
    from contextlib import ExitStack

    f32 = mybir.dt.float32
    f16 = mybir.dt.float16
    f32r = mybir.dt.float32r
    AX = mybir.AxisListType
    OP = mybir.AluOpType
    ACT = mybir.ActivationFunctionType

    def r(ap):
        return ap.bitcast(f32r)

    def mktile(pool, shape, tag, dtype=None):
        return pool.tile(shape, dtype or f32, name=tag, tag=tag)

    nc = bacc.Bacc(trn_type="TRN2", target_bir_lowering=False, num_devices=NCORES)

    x_ext = nc.declare_dram_parameter("x", [C, L], f32, isOutput=False)
    wq_ext = nc.declare_dram_parameter("wqt", [C, C], f32, isOutput=False)
    wk_ext = nc.declare_dram_parameter("wkt", [C, C], f32, isOutput=False)
    wv_ext = nc.declare_dram_parameter("wvt", [C, C], f32, isOutput=False)
    wp_ext = nc.declare_dram_parameter("wpt", [C, C], f32, isOutput=False)
    bq_ext = nc.declare_dram_parameter("bq", [C, 1], f32, isOutput=False)
    bk_ext = nc.declare_dram_parameter("bk", [C, 1], f32, isOutput=False)
    bv_ext = nc.declare_dram_parameter("bv", [C, 1], f32, isOutput=False)
    bp_ext = nc.declare_dram_parameter("bp", [C, 1], f32, isOutput=False)
    gnw_ext = nc.declare_dram_parameter("gnw", [C, 1], f32, isOutput=False)
    gnb_ext = nc.declare_dram_parameter("gnb", [C, 1], f32, isOutput=False)
    ones_ext = nc.declare_dram_parameter("ones", [P, 64], f32, isOutput=False)
    ind_ext = nc.declare_dram_parameter("ind", [C, G], f32, isOutput=False)
    indT_ext = nc.declare_dram_parameter("indT", [G, C], f32, isOutput=False)
    out_ext = nc.declare_dram_parameter("out", [C, DH], f32, isOutput=True)

    with tile.TileContext(nc) as tc, ExitStack() as top:
        # ---- pool A: kernel-long tiles -------------------------------
        pa = top.enter_context(tc.tile_pool(name="pa", bufs=1))
        wts = {}
        for nm, ext in (("wqt", wq_ext), ("wkt", wk_ext), ("wvt", wv_ext)):
            wts[nm] = []
            for ct in range(CT):
                t = mktile(pa, [P, C], f"{nm}{ct}")
                nc.gpsimd.dma_start(out=r(t[:]), in_=r(ext[ct * P:(ct + 1) * P, :]))
                wts[nm].append(t)
        # proj weight per head (base partition 0 for K=64 contraction)
        wpth = []
        for h in range(NH):
            t = mktile(pa, [HD, C], f"wpt{h}")
            nc.gpsimd.dma_start(out=r(t[:]), in_=r(wp_ext[h * HD:(h + 1) * HD, :]))
            wpth.append(t)
        bias = {}
        for nm, ext in (("bq", bq_ext), ("bk", bk_ext), ("bv", bv_ext),
                        ("bp", bp_ext), ("gnw", gnw_ext), ("gnb", gnb_ext)):
            bias[nm] = []
            for ct in range(CT):
                t = mktile(pa, [P, 1], f"{nm}{ct}")
                nc.sync.dma_start(out=t[:], in_=ext[ct * P:(ct + 1) * P, :])
                bias[nm].append(t)
        indt = []
        for ct in range(CT):
            t = mktile(pa, [P, G], f"ind{ct}")
            nc.sync.dma_start(out=r(t[:]), in_=r(ind_ext[ct * P:(ct + 1) * P, :]))
            indt.append(t)
        indTt = mktile(pa, [G, C], "indTt")
        nc.sync.dma_start(out=indTt[:], in_=indT_ext[:])
        ident = mktile(pa, [P, P], "ident")
        make_identity(nc, ident[:])
        ones_src = mktile(pa, [P, 64], "ones_src")
        nc.sync.dma_start(out=r(ones_src[:]), in_=r(ones_ext[:]))
        # attention output, one tile per head [64, DH] at base partition 0
        ao = [mktile(pa, [HD, DH], f"ao{h}") for h in range(NH)]

        # ---- pool B: normalized activations h ------------------------
        pb = top.enter_context(tc.tile_pool(name="pb", bufs=1))
        ht = [mktile(pb, [P, L], f"h{ct}") for ct in range(CT)]

        # ---- GroupNorm ----------------------------------------------
        with ExitStack() as ph:
            px = ph.enter_context(tc.tile_pool(name="px", bufs=1))
            pgs = ph.enter_context(tc.tile_pool(name="pgs", bufs=1))
            pgp = ph.enter_context(tc.tile_pool(name="pgp", bufs=1, space="PSUM"))
            xs = []
            for ct in range(CT):
                t = mktile(px, [P, L], f"x{ct}")
                for lc in range(LC):
                    sl = slice(lc * 512, (lc + 1) * 512)
                    nc.sync.dma_start(out=r(t[:, sl]),
                                      in_=r(x_ext[ct * P:(ct + 1) * P, sl]))
                xs.append(t)
            xsq = [mktile(px, [P, L], f"xsq{ct}") for ct in range(CT)]
            for ct in range(CT):
                for lc in range(LC):
                    sl = slice(lc * 512, (lc + 1) * 512)
                    nc.vector.tensor_mul(r(xsq[ct][:, sl]),
                                         xs[ct][:, sl], xs[ct][:, sl])
            ps_s = mktile(pgp, [G, 512], "ps_s")
            ps_q = mktile(pgp, [G, 512], "ps_q")
            n = 0
            for ct in range(CT):
                for lc in range(LC):
                    sl = slice(lc * 512, (lc + 1) * 512)
                    st, sp = (n == 0), (n == CT * LC - 1)
                    nc.tensor.matmul(ps_s[:], r(indt[ct][:]), r(xs[ct][:, sl]),
                                     start=st, stop=sp)
                    nc.tensor.matmul(ps_q[:], r(indt[ct][:]), r(xsq[ct][:, sl]),
                                     start=st, stop=sp)
                    n += 1
            sums = mktile(pgs, [G, 1], "sums")
            sumq = mktile(pgs, [G, 1], "sumq")
            nc.vector.reduce_sum(sums[:], ps_s[:], axis=AX.X)
            nc.vector.reduce_sum(sumq[:], ps_q[:], axis=AX.X)
            inv_n = 1.0 / float((C // G) * L)
            mrs = mktile(pgs, [G, 2], "mrs")      # col0 = mean, col1 = rstd
            var = mktile(pgs, [G, 1], "var")
            sqv = mktile(pgs, [G, 1], "sqv")
            nc.vector.tensor_scalar_mul(mrs[:, 0:1], sums[:], inv_n)
            nc.vector.tensor_scalar_mul(var[:], sumq[:], inv_n)
            nc.vector.tensor_mul(sqv[:], mrs[:, 0:1], mrs[:, 0:1])
            nc.vector.tensor_sub(var[:], var[:], sqv[:])
            eps_t = mktile(pgs, [G, 1], "eps")
            nc.vector.memset(eps_t[:], EPS)
            nc.scalar.activation(sqv[:], var[:], ACT.Ln, bias=eps_t[:])
            nc.scalar.activation(mrs[:, 1:2], sqv[:], ACT.Exp, scale=-0.5)
            # broadcast group stats to channels via PE: bc[c, :] = mrs[g(c), :]
            for ct in range(CT):
                bc_ps = mktile(pgp, [P, 2], f"bcps{ct}")
                nc.tensor.matmul(bc_ps[:], indTt[:, ct * P:(ct + 1) * P],
                                 mrs[:], start=True, stop=True)
                s_t = mktile(pgs, [P, 1], f"s{ct}")
                t_t = mktile(pgs, [P, 1], f"t{ct}")
                nc.vector.tensor_mul(s_t[:], bc_ps[:, 1:2], bias["gnw"][ct][:])
                nc.vector.tensor_mul(t_t[:], bc_ps[:, 0:1], s_t[:])
                nc.vector.tensor_sub(t_t[:], bias["gnb"][ct][:], t_t[:])
                nc.vector.tensor_scalar(r(ht[ct][:]), xs[ct][:], s_t[:], t_t[:],
                                        op0=OP.mult, op1=OP.add)

        # ---- pool C: q/k/VT (live through attention) -----------------
        pc = top.enter_context(tc.tile_pool(name="pc", bufs=1))
        qt = [mktile(pc, [P, DH], f"q{p}", dtype=f16) for p in range(CT)]
        kt = [mktile(pc, [P, L], f"k{p}", dtype=f16) for p in range(CT)]
        vt = [mktile(pc, [P, 65 * ECH], f"vt{h}", dtype=f16) for h in range(NH)]

        # ---- QKV + V^T, as chunk closures ---------------------------
        # Pair 0's chunks are emitted up front; pair 1's are deferred into
        # pair 0's first attention block so they execute under the
        # ScalarE-bound exp stream. All QKV/transpose PSUM goes through the
        # shared "aux" pool (also used by the softmax-denominator broadcast
        # and the projection).
        vpair = [mktile(pc, [P, L], f"v{p}") for p in range(CT)]

        def qkv_chunks(p, paux):
            work = []
            osl = slice(p * P, (p + 1) * P)
            streams = [("wqt", "bq", qt[p], DBLK),
                       ("wkt", "bk", kt[p], LC),
                       ("wvt", "bv", vpair[p], LC)]
            for wnm, bnm, dst, nchunks in streams:
                for cchunk in range(nchunks):
                    def go(wnm=wnm, bnm=bnm, dst=dst, cchunk=cchunk):
                        sl = slice(cchunk * 512, (cchunk + 1) * 512)
                        ps = paux.tile([P, 512], f32, name="aux", tag="aux")
                        for ct in range(CT):
                            nc.tensor.matmul(ps[:], r(wts[wnm][ct][:, osl]),
                                             r(ht[ct][:, sl]),
                                             start=(ct == 0), stop=(ct == CT - 1))
                        dst_ap = dst[:, sl]
                        if dst_ap.dtype == f32:
                            dst_ap = r(dst_ap)
                        nc.vector.tensor_scalar_add(dst_ap, ps[:], bias[bnm][p][:])
                    work.append(go)
            vt_work = []
            for ecg in range(ECH // 4):
                def go(p=p, ecg=ecg):
                    pst = paux.tile([P, 512], f32, name="aux", tag="aux")
                    for j in range(4):
                        esl = slice((ecg * 4 + j) * P, (ecg * 4 + j + 1) * P)
                        nc.tensor.transpose(pst[:, j * P:(j + 1) * P],
                                            vpair[p][:, esl], ident[:])
                    for h01 in range(2):
                        head = 2 * p + h01
                        outap = (vt[head][:, ecg * 260:(ecg + 1) * 260]
                                 .rearrange("p (a b) -> p a b", b=65)[:, :, 0:64])
                        inap = (pst[:].rearrange("p (a b) -> p a b", b=P)
                                [:, :, h01 * 64:(h01 + 1) * 64])
                        nc.vector.tensor_copy(out=outap, in_=inap)
                vt_work.append(go)
            # interleave: v chunk g then vT group g (v chunks are the last
            # LC entries of `work` before this point)
            vwork = work[-LC:]
            del work[-LC:]
            for g in range(LC):
                work.append(vwork[g])
                work.append(vt_work[g])
            for h01 in range(2):
                def go(p=p, h01=h01):
                    h = 2 * p + h01
                    ones_ap = (vt[h][:].rearrange("p (a b) -> p a b", b=65)
                               [:, :, 64:65])
                    nc.vector.tensor_copy(
                        out=ones_ap,
                        in_=ones_src[:, 0:ECH].rearrange("p a -> p a ()"))
                work.append(go)
            return work

        # ---- attention + per-db projection --------------------------
        # db is the outer loop so that projection for a finished d-block
        # overlaps the next d-block's attention; epilogues (softmax
        # normalize) are emitted deferred, a few ec-iterations into the
        # following (p, db) chunk, so their latency never stalls the PE
        # stream at chunk boundaries.
        with ExitStack() as ph:
            pe_s = ph.enter_context(tc.tile_pool(name="pes", bufs=3))
            pf = ph.enter_context(tc.tile_pool(name="pf", bufs=1))
            pfs = ph.enter_context(tc.tile_pool(name="pfs", bufs=3))
            psp = ph.enter_context(tc.tile_pool(name="psp", bufs=2, space="PSUM"))
            pap = ph.enter_context(tc.tile_pool(name="pap", bufs=2, space="PSUM"))
            paux = ph.enter_context(tc.tile_pool(name="paux", bufs=2, space="PSUM"))
            xres = []
            for ct in range(CT):
                t = mktile(pf, [P, DH], f"xr{ct}")
                nc.sync.dma_start(out=t[:], in_=x_ext[ct * P:(ct + 1) * P, 0:DH])
                xres.append(t)
            p0_work = qkv_chunks(0, paux)
            p1_work = qkv_chunks(1, paux)
            # early: p0's q/k chunks plus v0+vT0 and all ones-columns --
            # exactly what S[ec0]/PV[ec0] need (ones only depend on ones_src).
            # The rest is paced into the exp-bound ec loop: 2 closures/ec
            # while the backlog is large (so p1's q/k land before its first S),
            # then 1/ec so PE is not overloaded while ScalarE streams exps.
            n_qk = DBLK + LC
            early = p0_work[:n_qk + 2] + p0_work[-2:] + p1_work[-2:]
            for fn in early:
                fn()
            deferred = p0_work[n_qk + 2:-2] + p1_work[:-2]

            def epilogue_head(p, db, acc, h01):
                dsl = slice(db * 512, (db + 1) * 512)
                head = 2 * p + h01
                zc = mktile(pe_s, [65, 512], "zc")
                nc.vector.tensor_copy(out=r(zc[:]), in_=acc[h01][:])
                zb_ps = paux.tile([P, 512], f32, name="aux", tag="aux")
                nc.tensor.matmul(zb_ps[0:64, :], r(ones_src[64:65, 0:64]),
                                 r(zc[64:65, :]), start=True, stop=True)
                zb = mktile(pe_s, [64, 512], "zb")
                with nc.allow_low_precision("softmax denom, well conditioned"):
                    nc.vector.reciprocal_approx_fast(zb[:], zb_ps[0:64, :])
                nc.vector.tensor_mul(r(ao[head][:, dsl]),
                                     zc[0:64, :], zb[:])

            def proj_ot(db, ot):
                dsl = slice(db * 512, (db + 1) * 512)
                osl = slice(ot * P, (ot + 1) * P)
                ps = paux.tile([P, 512], f32, name="aux", tag="aux")
                for h in range(NH):
                    nc.tensor.matmul(ps[:], r(wpth[h][:, osl]),
                                     r(ao[h][:, dsl]),
                                     start=(h == 0), stop=(h == NH - 1))
                osb = mktile(pfs, [P, 512], "osb")
                nc.vector.scalar_tensor_tensor(
                    out=osb[:], in0=ps[:], scalar=bias["bp"][ot][:],
                    in1=xres[ot][:, dsl], op0=OP.add, op1=OP.add)
                nc.sync.dma_start(out=out_ext[osl, dsl], in_=osb[:])

            pending = []
            epi_unit = []
            for db in range(DBLK):
                for p in range(CT):
                    dsl = slice(db * 512, (db + 1) * 512)
                    acc = [mktile(pap, [65, 512], "acc") for _ in range(2)]
                    for ec in range(ECH):
                        esl = slice(ec * P, (ec + 1) * P)
                        sps = mktile(psp, [P, 1024], "sps")
                        for h01 in range(2):
                            hsl = slice(h01 * 64, (h01 + 1) * 64)
                            nc.tensor.matmul(sps[:, h01 * 512:(h01 + 1) * 512],
                                             kt[p][hsl, esl], qt[p][hsl, dsl],
                                             start=True, stop=True)
                        pt = mktile(pe_s, [P, 1024], "pt", dtype=f16)
                        nc.scalar.activation(pt[:], sps[:], ACT.Exp, scale=SCALE)
                        for h01 in range(2):
                            nc.tensor.matmul(
                                acc[h01][:],
                                vt[2 * p + h01][:, ec * 65:(ec + 1) * 65],
                                pt[:, h01 * 512:(h01 + 1) * 512],
                                start=(ec == 0), stop=(ec == ECH - 1))
                        if ec >= 2:
                            budget = 2 if len(deferred) > 26 else 1
                            for _ in range(budget):
                                if deferred:
                                    deferred.pop(0)()
                                elif pending:
                                    pending.pop(0)()
                                    break
                    for h01 in range(2):
                        pending.append(
                            lambda p=p, db=db, acc=acc, h01=h01:
                            epilogue_head(p, db, acc, h01))
                    if p == CT - 1:
                        for ot in range(CT):
                            pending.append(lambda db=db, ot=ot: proj_ot(db, ot))
            for fn in deferred + pending:
                fn()

    nc.compile()
    return nc


def _in_maps(inputs):
    x = np.asarray(inputs["x"], dtype=np.float32)
    gnw = np.ascontiguousarray(np.asarray(inputs["gn_w"], np.float32).reshape(C, 1))
    gnb = np.ascontiguousarray(np.asarray(inputs["gn_b"], np.float32).reshape(C, 1))
    wqt = np.ascontiguousarray(np.asarray(inputs["wq"], np.float32).T)
    wkt = np.ascontiguousarray(np.asarray(inputs["wk"], np.float32).T)
    wvt = np.ascontiguousarray(np.asarray(inputs["wv"], np.float32).T)
    wpt = np.ascontiguousarray(np.asarray(inputs["wp"], np.float32).T)
    bq = np.ascontiguousarray(np.asarray(inputs["bq"], np.float32).reshape(C, 1))
    bk = np.ascontiguousarray(np.asarray(inputs["bk"], np.float32).reshape(C, 1))
    bv = np.ascontiguousarray(np.asarray(inputs["bv"], np.float32).reshape(C, 1))
    bp = np.ascontiguousarray(np.asarray(inputs["bp"], np.float32).reshape(C, 1))
    ind = np.zeros((C, G), np.float32)
    ind[np.arange(C), np.arange(C) // (C // G)] = 1.0
    indT = np.ascontiguousarray(ind.T)
    ones = np.ones((P, 64), np.float32)
    common = dict(wqt=wqt, wkt=wkt, wvt=wvt, wpt=wpt, bq=bq, bk=bk, bv=bv,
                  bp=bp, gnw=gnw, gnb=gnb, ind=ind, indT=indT, ones=ones)
    maps = []
    for core in range(NCORES):
        b, half = core // 2, core % 2
        xb = np.ascontiguousarray(x[b].reshape(C, L))
        if half == 1:
            xb = np.ascontiguousarray(
                np.concatenate([xb[:, DH:], xb[:, :DH]], axis=1))
        maps.append(dict(common, x=xb))
    return maps


def kernel(**inputs) -> np.ndarray:
    from concourse.bass_utils import run_bass_kernel_spmd

    if "nc" not in _CACHE:
        _CACHE["nc"] = _build_nc()
    nc = _CACHE["nc"]
    res = run_bass_kernel_spmd(nc, _in_maps(inputs), core_ids=list(range(NCORES)))
    out = np.empty((B, C, L), np.float32)
    for core in range(NCORES):
        b, half = core // 2, core % 2
        out[b][:, half * DH:(half + 1) * DH] = res.results[core]["out"]
    return out.reshape(B, C, H, W)
